# revision 25
# baseline (speedup 1.0000x reference)
"""MoSRAH router kernel for 8 trn2 NeuronCores (Bass/Tile).

Strategy (data/sequence parallel per the sharding hint):
  - Flatten tokens (B*N = 16384) and shard 2048 tokens per core.
  - x is shipped as an fp16 hi/lo split (x == xhi + xlo to ~2^-22), host-
    permuted to [tau, partition(h%128), chunk(h//128), token] so each DMA
    descriptor is a long contiguous run per partition; the whole fp16 shard
    fits in SBUF, so every piece DMA is issued upfront as two pure FIFO
    streams (hi on the Sync HWDGE ring, lo on the ACT ring).
  - Matmul: fp32-grade logits from fp16 passes (hi*hi + lo*hi + hi*lo),
    accumulated in fp32 PSUM. The W hi/lo halves are stacked into one
    128-column stationary operand, so hi*hi and lo*hi run in a single
    matmul (output split across PSUM partitions 0-63 / 64-127); the two
    halves are summed after the PE transpose to [token, l] layout.
  - Top-8 via DVE max8/find_index8; expert histogram via match_replace +
    compare; probs = softmax over the 8 selected logits (the full softmax
    denominator cancels after renormalization).
  - Host: concat shards, sum the 8 per-core histograms, compute the two
    loss scalars (the all-reduce of a 64-vector).
"""

import sys

if "/opt/trn_rl_repo" not in sys.path:
    sys.path.insert(0, "/opt/trn_rl_repo")

from contextlib import ExitStack

import numpy as np

B, N, H, L, K = 4, 4096, 2048, 64, 8
N_CORES = 8
TC = (B * N) // N_CORES  # tokens per core = 2048
NTAU = 4                 # 512-token tiles per core
TT = 512
NG = 4                   # 128-token groups per tile
NGRP = NTAU * NG         # 16
HC = H // 128            # 16 h-chunks
BIG = 2.0e30
THRESH = 1.0e30

# x-DMA piece schedule (chunk ranges); tiny leading pieces on tile 0 so the
# first matmuls start as early as possible.
PIECES_T0 = [(0, 1), (1, 1), (2, 2), (4, 4), (8, 4), (12, 4)]
PIECES = [(0, 4), (4, 4), (8, 4), (12, 4)]

_CACHE = {}


def _pieces(tau):
    return PIECES_T0 if tau == 0 else PIECES


def _build_nc():
    import concourse.bass as bass
    import concourse.tile as tile
    from concourse import bacc, mybir

    f32 = mybir.dt.float32
    f16 = mybir.dt.float16
    u8 = mybir.dt.uint8
    u32 = mybir.dt.uint32

    nc = bacc.Bacc("TRN2", target_bir_lowering=False, debug=False,
                   num_devices=N_CORES)
    xhi_d = nc.declare_dram_parameter("xhi", [NTAU, 128, HC, TT], f16, isOutput=False)
    xlo_d = nc.declare_dram_parameter("xlo", [NTAU, 128, HC, TT], f16, isOutput=False)
    # stacked [whi | wlo]: cols 0-63 are W_hi, 64-127 are W_lo
    wst_d = nc.declare_dram_parameter("wst", [128, HC, 2 * L], f16, isOutput=False)
    bias = nc.declare_dram_parameter("bias", [L], f32, isOutput=False)
    mask = nc.declare_dram_parameter("mask", [128, NGRP], u8, isOutput=False)
    sel_o = nc.declare_dram_parameter("sel", [128, NGRP, K], u32, isOutput=True)
    prob_o = nc.declare_dram_parameter("probs", [128, NGRP, K], f32, isOutput=True)
    cnt_o = nc.declare_dram_parameter("cnt", [1, NG * L], f32, isOutput=True)

    with tile.TileContext(nc) as tc, ExitStack() as ctx:
        const = ctx.enter_context(tc.tile_pool(name="const", bufs=1))
        wk = ctx.enter_context(tc.tile_pool(name="wk", bufs=2))
        ps_lt = ctx.enter_context(tc.tile_pool(name="ps_lt", bufs=2, space="PSUM"))
        ps_tp = ctx.enter_context(tc.tile_pool(name="ps_tp", bufs=2, space="PSUM"))
        ps_cnt = ctx.enter_context(tc.tile_pool(name="ps_cnt", bufs=1, space="PSUM"))

        # --- constants; W-chunks 0-3 lead the Sync ring, the rest follows
        # the first x piece so matmuls can start early --------------------
        wst = const.tile([128, HC, 2 * L], f16)
        nc.sync.dma_start(wst[:][:, 0:4, :], wst_d[:][:, 0:4, :])

        b_sb = const.tile([64, 1], f32)
        nc.scalar.dma_start(b_sb[:], bias[:].rearrange("(a b) -> a b", b=1))
        m_u8 = const.tile([128, NGRP], u8)
        nc.scalar.dma_start(m_u8[:], mask[:])
        m_f = const.tile([128, NGRP], f32)
        nc.vector.tensor_copy(m_f[:], m_u8[:])

        # --- x piece DMAs, all issued upfront in consumption order --------
        xhi_t, xlo_t, cmap = {}, {}, {}
        first = True
        for tau in range(NTAU):
            for pi, (c0, cl) in enumerate(_pieces(tau)):
                cs = slice(c0, c0 + cl)
                th = const.tile([128, cl, TT], f16, tag=f"xhi_{tau}_{pi}")
                nc.sync.dma_start(th[:], xhi_d[:][tau, :, cs, :])
                tl = const.tile([128, cl, TT], f16, tag=f"xlo_{tau}_{pi}")
                nc.scalar.dma_start(tl[:], xlo_d[:][tau, :, cs, :])
                xhi_t[tau, pi] = th
                xlo_t[tau, pi] = tl
                for c in range(c0, c0 + cl):
                    cmap[tau, c] = (pi, c - c0)
                if first:
                    nc.sync.dma_start(wst[:][:, 4:HC, :], wst_d[:][:, 4:HC, :])
                    first = False

        ones64 = const.tile([64, 64], f32)
        nc.gpsimd.memset(ones64[:], 1.0)
        ident = const.tile([64, 64], f32)
        nc.gpsimd.affine_select(
            ident[:], ones64[:], pattern=[[-1, 64]],
            compare_op=mybir.AluOpType.is_equal, fill=0.0,
            base=0, channel_multiplier=1,
        )
        ones128 = const.tile([128, 1], f32)
        nc.gpsimd.memset(ones128[:], 1.0)

        acc = const.tile([128, NG, L], f32)
        nc.vector.memset(acc[:], 0.0)

        vals = const.tile([128, NGRP, K], f32)
        idx = const.tile([128, NGRP, K], u32)
        pr_sb = const.tile([128, NGRP, K], f32)

        # --- main loop over 512-token tiles -------------------------------
        for tau in range(NTAU):
            lt = ps_lt.tile([128, TT], f32)
            for c in range(HC):
                pi, co = cmap[tau, c]
                xh = xhi_t[tau, pi][:][:, co, :]
                xl = xlo_t[tau, pi][:][:, co, :]
                # one matmul: rows 0-63 += whi.x_hi, rows 64-127 += wlo.x_hi;
                # plus whi.x_lo accumulated onto rows 0-63. On the last chunk
                # the full-width matmul goes last so it closes the group.
                if c < HC - 1:
                    nc.tensor.matmul(
                        lt[:], lhsT=wst[:][:, c, :], rhs=xh,
                        start=(c == 0), stop=False,
                    )
                    nc.tensor.matmul(
                        lt[:][0:64, :], lhsT=wst[:][:, c, 0:L], rhs=xl,
                        start=False, stop=False,
                    )
                else:
                    nc.tensor.matmul(
                        lt[:][0:64, :], lhsT=wst[:][:, c, 0:L], rhs=xl,
                        start=False, stop=False,
                    )
                    nc.tensor.matmul(
                        lt[:], lhsT=wst[:][:, c, :], rhs=xh,
                        start=False, stop=True,
                    )

            # the two PSUM partition halves -> SBUF (bias folded into A)
            yta = wk.tile([64, TT], f32, tag="yta")
            nc.vector.tensor_scalar_add(yta[:], lt[:][0:64, :], b_sb[:][:, 0:1])
            ytb = wk.tile([64, TT], f32, tag="ytb")
            nc.scalar.copy(ytb[:], lt[:][64:128, :])

            # transpose both halves to [token, l]; the second transpose
            # accumulates onto the first in PSUM (it's just a matmul)
            tp = ps_tp.tile([128, NG * L], f32)
            for gl in range(NG):
                sl = tp[:][:, gl * L:(gl + 1) * L]
                nc.tensor.matmul(
                    sl, lhsT=yta[:][:, gl * 128:(gl + 1) * 128],
                    rhs=ident[:], is_transpose=True, start=True, stop=False,
                )
                nc.tensor.matmul(
                    sl, lhsT=ytb[:][:, gl * 128:(gl + 1) * 128],
                    rhs=ident[:], is_transpose=True, start=False, stop=True,
                )

            y_sb = wk.tile([128, NG, L], f32, tag="ysb")
            nc.scalar.copy(y_sb[:].rearrange("p a b -> p (a b)"), tp[:])

            rep = wk.tile([128, NG, L], f32, tag="rep")
            for gl in range(NG):
                g = tau * NG + gl
                nc.vector.max(vals[:][:, g], y_sb[:][:, gl])
                nc.vector.max_index(idx[:][:, g], vals[:][:, g], y_sb[:][:, gl])
                nc.vector.match_replace(rep[:][:, gl], vals[:][:, g], y_sb[:][:, gl], BIG)

            # histogram: (rep >= THRESH) * active_mask, accumulated
            m_sl = m_f[:][:, tau * NG:(tau + 1) * NG]
            m_bc = bass.AP(m_sl.tensor, m_sl.offset, m_sl.ap + [[0, L]])
            ohm = wk.tile([128, NG, L], f32, tag="ohm")
            nc.vector.scalar_tensor_tensor(
                ohm[:], rep[:], THRESH, m_bc,
                op0=mybir.AluOpType.is_ge, op1=mybir.AluOpType.mult,
            )
            nc.vector.tensor_add(acc[:], acc[:], ohm[:])

            # softmax over the 8 selected logits for this tile's 4 groups
            gs = slice(tau * NG, (tau + 1) * NG)
            v_sl = vals[:][:, gs, :]
            vmax = v_sl[:, :, 0:1].broadcast_to((128, NG, K))
            dtl = wk.tile([128, NG, K], f32, tag="dtl")
            nc.vector.tensor_sub(dtl[:], v_sl, vmax)
            etl = wk.tile([128, NG, K], f32, tag="etl")
            nc.scalar.activation(etl[:], dtl[:], mybir.ActivationFunctionType.Exp)
            stl = wk.tile([128, NG], f32, tag="stl")
            nc.vector.reduce_sum(stl[:], etl[:], axis=mybir.AxisListType.X)
            rtl = wk.tile([128, NG], f32, tag="rtl")
            nc.vector.reciprocal(rtl[:], stl[:])
            r_ap = rtl[:]
            r_bc = bass.AP(r_ap.tensor, r_ap.offset, r_ap.ap + [[0, K]])
            nc.vector.tensor_mul(pr_sb[:][:, gs, :], etl[:], r_bc)

        # --- expert counts: reduce over partitions via ones-matmul --------
        cnt_ps = ps_cnt.tile([1, NG * L], f32)
        nc.tensor.matmul(
            cnt_ps[:], lhsT=ones128[:],
            rhs=acc[:].rearrange("p a b -> p (a b)"),
            start=True, stop=True,
        )
        cnt_sb = const.tile([1, NG * L], f32)
        nc.vector.tensor_copy(cnt_sb[:], cnt_ps[:])

        # --- outputs: after every x piece has been issued on both rings ---
        nc.sync.dma_start(sel_o[:], idx[:])
        nc.sync.dma_start(prob_o[:], pr_sb[:])
        nc.sync.dma_start(cnt_o[:], cnt_sb[:])

    nc.finalize()
    return nc


def _get_nc():
    if "nc" not in _CACHE:
        _CACHE["nc"] = _build_nc()
    return _CACHE["nc"]


def _make_in_maps(x, W_r, expert_bias, active_mask):
    xf = np.asarray(x, dtype=np.float32).reshape(B * N, H)
    wT = np.asarray(W_r, dtype=np.float32).T  # [H, L]
    wperm = np.ascontiguousarray(wT.reshape(HC, 128, L).transpose(1, 0, 2))
    whi = wperm.astype(np.float16)
    wlo = (wperm - whi.astype(np.float32)).astype(np.float16)
    wst = np.ascontiguousarray(np.concatenate([whi, wlo], axis=-1))
    bias = np.ascontiguousarray(np.asarray(expert_bias, dtype=np.float32))
    mf = np.asarray(active_mask).reshape(B * N).astype(np.uint8)
    in_maps = []
    for i in range(N_CORES):
        shard = xf[i * TC:(i + 1) * TC]
        # [tau, t, c, p] -> [tau, p, c, t]
        xtile = np.ascontiguousarray(
            shard.reshape(NTAU, TT, HC, 128).transpose(0, 3, 2, 1)
        )
        xhi = xtile.astype(np.float16)
        xlo = (xtile - xhi.astype(np.float32)).astype(np.float16)
        mperm = np.ascontiguousarray(
            mf[i * TC:(i + 1) * TC].reshape(NGRP, 128).T
        )
        in_maps.append(dict(
            xhi=xhi,
            xlo=xlo,
            wst=wst,
            bias=bias,
            mask=mperm,
        ))
    return in_maps


def _assemble(results, active_mask):
    sels, ps = [], []
    cnt_total = np.zeros(L, np.float32)
    for r in results:
        s = r["sel"].reshape(128, NTAU, NG, K).transpose(1, 2, 0, 3).reshape(TC, K)
        p = r["probs"].reshape(128, NTAU, NG, K).transpose(1, 2, 0, 3).reshape(TC, K)
        sels.append(s.astype(np.int32))
        ps.append(p.astype(np.float32))
        cnt_total += r["cnt"].reshape(NG, L).sum(0, dtype=np.float32)
    sel_full = np.concatenate(sels, 0).reshape(B, N, K)
    probs_full = np.concatenate(ps, 0).reshape(B, N, K)
    am = np.asarray(active_mask).astype(np.float32)
    denom = np.float32(am.sum(dtype=np.float32) * K)
    freqs = (cnt_total / denom).astype(np.float32)
    inv_L = np.float32(1.0 / L)
    lbl = np.float32(L * np.sum((freqs - inv_L) ** 2, dtype=np.float32))
    vio = np.float32(L * np.max(freqs - inv_L))
    return sel_full, probs_full, lbl, vio


def run(x, W_r, expert_bias, active_mask, trace=False, **kw):
    from concourse.bass_utils import run_bass_kernel_spmd

    nc = _get_nc()
    in_maps = _make_in_maps(x, W_r, expert_bias, active_mask)
    res = run_bass_kernel_spmd(nc, in_maps, list(range(N_CORES)), trace=trace, **kw)
    return _assemble(res.results, active_mask), res


def kernel(x, W_r, expert_bias, active_mask):
    return run(x, W_r, expert_bias, active_mask)[0]


# revision 27
# speedup vs baseline: 1.1536x; 1.1536x over previous
"""MoSRAH router kernel for 8 trn2 NeuronCores (Bass/Tile).

Strategy (data/sequence parallel per the sharding hint):
  - Flatten tokens (B*N = 16384) and shard 2048 tokens per core.
  - x is shipped as an fp16 hi/lo split (x == xhi + xlo to ~2^-22), host-
    permuted to [tau, partition(h%128), chunk(h//128), token] so each DMA
    descriptor is a long contiguous run per partition; the whole fp16 shard
    fits in SBUF, so every piece DMA is issued upfront as two pure FIFO
    streams (hi on the Sync HWDGE ring, lo on the ACT ring).
  - Matmul: fp32-grade logits from fp16 passes (hi*hi + lo*hi + hi*lo),
    accumulated in fp32 PSUM. The W hi/lo halves are stacked into one
    128-column stationary operand, so hi*hi and lo*hi run in a single
    matmul (output split across PSUM partitions 0-63 / 64-127); the two
    halves are summed after the PE transpose to [token, l] layout.
  - Top-8 via DVE max8/find_index8; expert histogram via match_replace +
    compare; probs = softmax over the 8 selected logits (the full softmax
    denominator cancels after renormalization).
  - Host: concat shards, sum the 8 per-core histograms, compute the two
    loss scalars (the all-reduce of a 64-vector).
"""

import sys

if "/opt/trn_rl_repo" not in sys.path:
    sys.path.insert(0, "/opt/trn_rl_repo")

from contextlib import ExitStack

import numpy as np

B, N, H, L, K = 4, 4096, 2048, 64, 8
N_CORES = 8
TC = (B * N) // N_CORES  # tokens per core = 2048
NTAU = 4                 # 512-token tiles per core
TT = 512
NG = 4                   # 128-token groups per tile
NGRP = NTAU * NG         # 16
HC = H // 128            # 16 h-chunks
BIG = 2.0e30
THRESH = 1.0e30

# x-DMA piece schedule (chunk ranges); tiny leading pieces on tile 0 so the
# first matmuls start as early as possible.
PIECES_T0 = [(0, 1), (1, 1), (2, 2), (4, 4), (8, 4), (12, 4)]
PIECES = [(0, 4), (4, 4), (8, 4), (12, 4)]

_CACHE = {}


def _pieces(tau):
    return PIECES_T0 if tau == 0 else PIECES


def _build_nc():
    import concourse.bass as bass
    import concourse.tile as tile
    from concourse import bacc, mybir

    f32 = mybir.dt.float32
    f16 = mybir.dt.float16
    u8 = mybir.dt.uint8
    u32 = mybir.dt.uint32

    nc = bacc.Bacc("TRN2", target_bir_lowering=False, debug=False,
                   num_devices=N_CORES)
    xhi_d = nc.declare_dram_parameter("xhi", [NTAU, 128, HC, TT], f16, isOutput=False)
    xlo_d = nc.declare_dram_parameter("xlo", [NTAU, 128, HC, TT], f16, isOutput=False)
    # stacked [whi | wlo]: cols 0-63 are W_hi, 64-127 are W_lo
    wst_d = nc.declare_dram_parameter("wst", [128, HC, 2 * L], f16, isOutput=False)
    bias = nc.declare_dram_parameter("bias", [L], f32, isOutput=False)
    mask = nc.declare_dram_parameter("mask", [128, NGRP], u8, isOutput=False)
    sel_o = nc.declare_dram_parameter("sel", [128, NGRP, K], u32, isOutput=True)
    prob_o = nc.declare_dram_parameter("probs", [128, NGRP, K], f32, isOutput=True)
    cnt_o = nc.declare_dram_parameter("cnt", [1, NG * L], f32, isOutput=True)

    with tile.TileContext(nc) as tc, ExitStack() as ctx:
        const = ctx.enter_context(tc.tile_pool(name="const", bufs=1))
        wk = ctx.enter_context(tc.tile_pool(name="wk", bufs=2))
        ps_lt = ctx.enter_context(tc.tile_pool(name="ps_lt", bufs=2, space="PSUM"))
        ps_tp = ctx.enter_context(tc.tile_pool(name="ps_tp", bufs=2, space="PSUM"))
        ps_cnt = ctx.enter_context(tc.tile_pool(name="ps_cnt", bufs=1, space="PSUM"))

        # --- constants; W-chunks 0-3 lead the Sync ring, the rest follows
        # the first x piece so matmuls can start early --------------------
        wst = const.tile([128, HC, 2 * L], f16)
        nc.sync.dma_start(wst[:][:, 0:4, :], wst_d[:][:, 0:4, :])

        b_sb = const.tile([64, 1], f32)
        nc.scalar.dma_start(b_sb[:], bias[:].rearrange("(a b) -> a b", b=1))
        m_u8 = const.tile([128, NGRP], u8)
        nc.scalar.dma_start(m_u8[:], mask[:])
        m_f = const.tile([128, NGRP], f32)
        nc.vector.tensor_copy(m_f[:], m_u8[:])

        # --- x piece DMAs, all issued upfront in consumption order --------
        xhi_t, xlo_t, cmap = {}, {}, {}
        first = True
        for tau in range(NTAU):
            for pi, (c0, cl) in enumerate(_pieces(tau)):
                cs = slice(c0, c0 + cl)
                th = const.tile([128, cl, TT], f16, tag=f"xhi_{tau}_{pi}")
                nc.sync.dma_start(th[:], xhi_d[:][tau, :, cs, :])
                tl = const.tile([128, cl, TT], f16, tag=f"xlo_{tau}_{pi}")
                nc.scalar.dma_start(tl[:], xlo_d[:][tau, :, cs, :])
                xhi_t[tau, pi] = th
                xlo_t[tau, pi] = tl
                for c in range(c0, c0 + cl):
                    cmap[tau, c] = (pi, c - c0)
                if first:
                    nc.sync.dma_start(wst[:][:, 4:HC, :], wst_d[:][:, 4:HC, :])
                    first = False

        ones64 = const.tile([64, 64], f32)
        nc.gpsimd.memset(ones64[:], 1.0)
        ident = const.tile([64, 64], f32)
        nc.gpsimd.affine_select(
            ident[:], ones64[:], pattern=[[-1, 64]],
            compare_op=mybir.AluOpType.is_equal, fill=0.0,
            base=0, channel_multiplier=1,
        )
        ones128 = const.tile([128, 1], f32)
        nc.gpsimd.memset(ones128[:], 1.0)

        acc = const.tile([128, NG, L], f32)
        nc.vector.memset(acc[:], 0.0)

        vals = const.tile([128, NGRP, K], f32)
        idx = const.tile([128, NGRP, K], u32)
        pr_sb = const.tile([128, NGRP, K], f32)

        # --- main loop over 512-token tiles -------------------------------
        for tau in range(NTAU):
            lt = ps_lt.tile([128, TT], f32)
            for c in range(HC):
                pi, co = cmap[tau, c]
                xh = xhi_t[tau, pi][:][:, co, :]
                xl = xlo_t[tau, pi][:][:, co, :]
                # one matmul: rows 0-63 += whi.x_hi, rows 64-127 += wlo.x_hi;
                # plus whi.x_lo accumulated onto rows 0-63. On the last chunk
                # the full-width matmul goes last so it closes the group.
                if c < HC - 1:
                    nc.tensor.matmul(
                        lt[:], lhsT=wst[:][:, c, :], rhs=xh,
                        start=(c == 0), stop=False,
                    )
                    nc.tensor.matmul(
                        lt[:][0:64, :], lhsT=wst[:][:, c, 0:L], rhs=xl,
                        start=False, stop=False,
                    )
                else:
                    nc.tensor.matmul(
                        lt[:][0:64, :], lhsT=wst[:][:, c, 0:L], rhs=xl,
                        start=False, stop=False,
                    )
                    nc.tensor.matmul(
                        lt[:], lhsT=wst[:][:, c, :], rhs=xh,
                        start=False, stop=True,
                    )

            # the two PSUM partition halves -> SBUF (bias folded into A).
            # Both on DVE: the ACT queue is full of xlo DMA issues that
            # backpressure on the HW ring — compute there would stall PE.
            yta = wk.tile([64, TT], f32, tag="yta")
            nc.vector.tensor_scalar_add(yta[:], lt[:][0:64, :], b_sb[:][:, 0:1])
            ytb = wk.tile([64, TT], f32, tag="ytb")
            nc.vector.tensor_copy(ytb[:], lt[:][64:128, :])

            # transpose both halves to [token, l]; the second transpose
            # accumulates onto the first in PSUM (it's just a matmul)
            tp = ps_tp.tile([128, NG * L], f32)
            for gl in range(NG):
                sl = tp[:][:, gl * L:(gl + 1) * L]
                nc.tensor.matmul(
                    sl, lhsT=yta[:][:, gl * 128:(gl + 1) * 128],
                    rhs=ident[:], is_transpose=True, start=True, stop=False,
                )
                nc.tensor.matmul(
                    sl, lhsT=ytb[:][:, gl * 128:(gl + 1) * 128],
                    rhs=ident[:], is_transpose=True, start=False, stop=True,
                )

            y_sb = wk.tile([128, NG, L], f32, tag="ysb")
            nc.vector.tensor_copy(y_sb[:].rearrange("p a b -> p (a b)"), tp[:])

            rep = wk.tile([128, NG, L], f32, tag="rep")
            for gl in range(NG):
                g = tau * NG + gl
                nc.vector.max(vals[:][:, g], y_sb[:][:, gl])
                nc.vector.max_index(idx[:][:, g], vals[:][:, g], y_sb[:][:, gl])
                nc.vector.match_replace(rep[:][:, gl], vals[:][:, g], y_sb[:][:, gl], BIG)

            # histogram: (rep >= THRESH) * active_mask, accumulated
            m_sl = m_f[:][:, tau * NG:(tau + 1) * NG]
            m_bc = bass.AP(m_sl.tensor, m_sl.offset, m_sl.ap + [[0, L]])
            ohm = wk.tile([128, NG, L], f32, tag="ohm")
            nc.vector.scalar_tensor_tensor(
                ohm[:], rep[:], THRESH, m_bc,
                op0=mybir.AluOpType.is_ge, op1=mybir.AluOpType.mult,
            )
            nc.vector.tensor_add(acc[:], acc[:], ohm[:])

            # softmax over the 8 selected logits for this tile's 4 groups
            gs = slice(tau * NG, (tau + 1) * NG)
            v_sl = vals[:][:, gs, :]
            vmax = v_sl[:, :, 0:1].broadcast_to((128, NG, K))
            dtl = wk.tile([128, NG, K], f32, tag="dtl")
            nc.vector.tensor_sub(dtl[:], v_sl, vmax)
            etl = wk.tile([128, NG, K], f32, tag="etl")
            nc.scalar.activation(etl[:], dtl[:], mybir.ActivationFunctionType.Exp)
            stl = wk.tile([128, NG], f32, tag="stl")
            nc.vector.reduce_sum(stl[:], etl[:], axis=mybir.AxisListType.X)
            rtl = wk.tile([128, NG], f32, tag="rtl")
            nc.vector.reciprocal(rtl[:], stl[:])
            r_ap = rtl[:]
            r_bc = bass.AP(r_ap.tensor, r_ap.offset, r_ap.ap + [[0, K]])
            nc.vector.tensor_mul(pr_sb[:][:, gs, :], etl[:], r_bc)

        # --- expert counts: reduce over partitions via ones-matmul --------
        cnt_ps = ps_cnt.tile([1, NG * L], f32)
        nc.tensor.matmul(
            cnt_ps[:], lhsT=ones128[:],
            rhs=acc[:].rearrange("p a b -> p (a b)"),
            start=True, stop=True,
        )
        cnt_sb = const.tile([1, NG * L], f32)
        nc.vector.tensor_copy(cnt_sb[:], cnt_ps[:])

        # --- outputs: after every x piece has been issued on both rings ---
        nc.sync.dma_start(sel_o[:], idx[:])
        nc.sync.dma_start(prob_o[:], pr_sb[:])
        nc.sync.dma_start(cnt_o[:], cnt_sb[:])

    nc.finalize()
    return nc


def _get_nc():
    if "nc" not in _CACHE:
        _CACHE["nc"] = _build_nc()
    return _CACHE["nc"]


def _make_in_maps(x, W_r, expert_bias, active_mask):
    xf = np.asarray(x, dtype=np.float32).reshape(B * N, H)
    wT = np.asarray(W_r, dtype=np.float32).T  # [H, L]
    wperm = np.ascontiguousarray(wT.reshape(HC, 128, L).transpose(1, 0, 2))
    whi = wperm.astype(np.float16)
    wlo = (wperm - whi.astype(np.float32)).astype(np.float16)
    wst = np.ascontiguousarray(np.concatenate([whi, wlo], axis=-1))
    bias = np.ascontiguousarray(np.asarray(expert_bias, dtype=np.float32))
    mf = np.asarray(active_mask).reshape(B * N).astype(np.uint8)
    in_maps = []
    for i in range(N_CORES):
        shard = xf[i * TC:(i + 1) * TC]
        # [tau, t, c, p] -> [tau, p, c, t]
        xtile = np.ascontiguousarray(
            shard.reshape(NTAU, TT, HC, 128).transpose(0, 3, 2, 1)
        )
        xhi = xtile.astype(np.float16)
        xlo = (xtile - xhi.astype(np.float32)).astype(np.float16)
        mperm = np.ascontiguousarray(
            mf[i * TC:(i + 1) * TC].reshape(NGRP, 128).T
        )
        in_maps.append(dict(
            xhi=xhi,
            xlo=xlo,
            wst=wst,
            bias=bias,
            mask=mperm,
        ))
    return in_maps


def _assemble(results, active_mask):
    sels, ps = [], []
    cnt_total = np.zeros(L, np.float32)
    for r in results:
        s = r["sel"].reshape(128, NTAU, NG, K).transpose(1, 2, 0, 3).reshape(TC, K)
        p = r["probs"].reshape(128, NTAU, NG, K).transpose(1, 2, 0, 3).reshape(TC, K)
        sels.append(s.astype(np.int32))
        ps.append(p.astype(np.float32))
        cnt_total += r["cnt"].reshape(NG, L).sum(0, dtype=np.float32)
    sel_full = np.concatenate(sels, 0).reshape(B, N, K)
    probs_full = np.concatenate(ps, 0).reshape(B, N, K)
    am = np.asarray(active_mask).astype(np.float32)
    denom = np.float32(am.sum(dtype=np.float32) * K)
    freqs = (cnt_total / denom).astype(np.float32)
    inv_L = np.float32(1.0 / L)
    lbl = np.float32(L * np.sum((freqs - inv_L) ** 2, dtype=np.float32))
    vio = np.float32(L * np.max(freqs - inv_L))
    return sel_full, probs_full, lbl, vio


def run(x, W_r, expert_bias, active_mask, trace=False, **kw):
    from concourse.bass_utils import run_bass_kernel_spmd

    nc = _get_nc()
    in_maps = _make_in_maps(x, W_r, expert_bias, active_mask)
    res = run_bass_kernel_spmd(nc, in_maps, list(range(N_CORES)), trace=trace, **kw)
    return _assemble(res.results, active_mask), res


def kernel(x, W_r, expert_bias, active_mask):
    return run(x, W_r, expert_bias, active_mask)[0]
